# revision 4
# baseline (speedup 1.0000x reference)
"""Trainium2 Bass kernel for nn_BayesianClassifier (MC-dropout 1x1-conv).

Structure: masks fold into per-sample weights (einsum(f*m, W) ==
einsum(f, W*m)); batch data-parallel over 8 cores; per chunk of 304
hw-columns, 10 fp16 matmuls (5 groups x 2 c-halves) -> PSUM [128,5,304];
one merged sigmoid ACT per chunk with per-partition bias (scalar engine
is the bottleneck: 5 x 40000 cols at 1 elem/cycle/lane).

Key optimizations vs the earlier baseline:
  - Weight walls are built ON DEVICE: host sends W^T [256,14] and the
    per-(sample,channel) scale [256,40] (27KB instead of 328KB of
    pre-expanded walls), and two broadcast tensor_tensor ops expand
    wall[c, g, s, k] = wt[c,k] * scale[c, g*8+s].  Walls become ready
    ~3us earlier and stop stealing startup DMA bandwidth from features.
  - Selector compressed to a sliding-window tile [128, 240]:
    sel_i = base[:, 112-14*i : 240-14*i] (61KB instead of 295KB).
  - SEL_LAG 7 -> 3, small supertiles at BOTH ends ([2,2,3,4,6] ... [4,3])
    so the final totals copies+DMAs mostly overlap the sigmoid stream;
    the last two output DMAs split across the sync and scalar queues.
  - Feature DMA queues balanced: f0 alternates sync/scalar by supertile
    parity, f1 rides gpsimd (st>=3); per-supertile totals stream out on
    the sync/scalar queue opposite to the next feature load.
  - Logit epilogue on host (from v3): kernel streams raw totals
    t = sum_s sigmoid() (fp32); kernel() computes log(t) - log(40-t).
"""

import numpy as np

B, C, H, W = 8, 256, 200, 200
S, K = 40, 14
HW = H * W
GROUPS = 5
SPG = 8
M = SPG * K  # 112
MP = 128
CHUNK = 304
CHUNK0 = 176  # 176 + 131*304 = 40000
BANK_F32 = 512
SUPER = 9
M2 = SUPER * K  # 126
SELW = 240  # sliding selector window width
NCORES = 8

_CACHE = {}


def _st_sizes():
    return [2, 2, 2, 3, 4, 7] + [SUPER] * 12 + [3, 2]


def _chunk_layout(hw_total):
    chunks = [64, 112] + [CHUNK] * ((hw_total - CHUNK0) // CHUNK)
    assert sum(chunks) == hw_total
    sizes = _st_sizes()
    assert sum(sizes) == len(chunks)
    sts = []
    idx = 0
    off = 0
    for n in sizes:
        ws = chunks[idx : idx + n]
        sts.append((off, ws))
        off += sum(ws)
        idx += n
    return sts


def _bank_windows(g, cw):
    lo, hi = g * CHUNK, g * CHUNK + cw
    cuts = [lo]
    b = (lo // BANK_F32 + 1) * BANK_F32
    while b < hi:
        cuts.append(b)
        b += BANK_F32
    cuts.append(hi)
    return [(a - lo, b2 - a) for a, b2 in zip(cuts, cuts[1:])]


def _build_program(hw_total):
    import concourse.bass as bass
    import concourse.bacc as bacc
    import concourse.tile as tile
    import concourse.mybir as mybir

    dt = mybir.dt
    f16, f32 = dt.float16, dt.float32

    nc = bacc.Bacc("TRN2", target_bir_lowering=False, debug=False)

    fh_d = nc.dram_tensor("fh", [C, hw_total], f16, kind="ExternalInput")
    wsc_d = nc.dram_tensor("wsc", [C, K + S], f16, kind="ExternalInput")
    sel_d = nc.dram_tensor("sel", [MP, SELW], f16, kind="ExternalInput")
    bias_d = nc.dram_tensor("biasv", [MP, 1], f32, kind="ExternalInput")

    sts = _chunk_layout(hw_total)
    n_st = len(sts)
    out_d = nc.dram_tensor("out", [M2, n_st * CHUNK], f32, kind="ExternalOutput")

    with tile.TileContext(nc) as tc:
        with (
            tc.tile_pool(name="const", bufs=1) as constp,
            tc.tile_pool(name="fpool", bufs=4) as fpool,
            tc.tile_pool(name="sigp", bufs=7) as sigp,
            tc.tile_pool(name="treep", bufs=7) as treep,
            tc.tile_pool(name="totsb", bufs=1) as totsb,
            tc.tile_pool(name="psl", bufs=2, space=bass.MemorySpace.PSUM) as psl,
            tc.tile_pool(name="pst", bufs=2, space=bass.MemorySpace.PSUM) as pst,
        ):
            warm = constp.tile([128, 384], f16)
            # memset on the idle vector queue: its preamble ends ~5.8us vs
            # gpsimd ~7.2us, so PE warmups start ~1us earlier and HAM is
            # warm before the first real chunk
            nc.vector.memset(warm[:], 0.0)

            off0, widths0 = sts[0]
            w_st0 = sum(widths0)
            f0_first = fpool.tile(
                [128, w_st0], f16, tag="f0", padded_shape=[128, SUPER * CHUNK]
            )
            f1_first = fpool.tile(
                [128, w_st0], f16, tag="f1", padded_shape=[128, SUPER * CHUNK]
            )
            # wall layout: [c, g, s_local*14+k] padded to 128 cols per group
            wall0 = constp.tile([128, GROUPS, MP], f16)
            wall1 = constp.tile([128, GROUPS, MP], f16)
            wsc0 = constp.tile([128, K + S], f16)
            wsc1 = constp.tile([128, K + S], f16)
            sel_s = constp.tile([MP, SELW], f16)
            bias_s = constp.tile([MP, 1], f32)
            czero = constp.tile([MP, 1], f32)
            scratch = constp.tile([MP, 1], f32)
            nc.vector.memset(czero[:], 0.0)
            nc.gpsimd.memset(wall0[:], 0.0)
            nc.gpsimd.memset(wall1[:], 0.0)

            # tiny wall ingredients first, then st0 features (176 cols)
            nc.sync.dma_start(wsc0[:], wsc_d[0:128, :])
            nc.scalar.dma_start(wsc1[:], wsc_d[128:256, :])
            nc.sync.dma_start(f0_first[:], fh_d[0:128, off0 : off0 + w_st0])
            nc.scalar.dma_start(f1_first[:], fh_d[128:256, off0 : off0 + w_st0])
            nc.gpsimd.dma_start(bias_s[:], bias_d[:])
            nc.gpsimd.dma_start(sel_s[:], sel_d[:])

            # hoist the sigmoid ACT_TABLE_LOAD off the critical path
            Sig = mybir.ActivationFunctionType.Sigmoid
            nc.scalar.activation(scratch[:], czero[:], Sig, bias=czero[:])

            # build walls on DVE: wall[c, g, s*14+k] = wt[c,k] * scale[c, g*8+s]
            for wall, wsc in ((wall0, wsc0), (wall1, wsc1)):
                live = wall[:, :, 0 : SPG * K]  # [128, 5, 112]
                wt_b = (
                    wsc[:, 0:K]
                    .unsqueeze(1)
                    .unsqueeze(2)
                    .broadcast_to([128, GROUPS, SPG, K])
                )
                sc_b = (
                    wsc[:, K : K + S]
                    .rearrange("p (g s) -> p g s", g=GROUPS)
                    .unsqueeze(3)
                    .broadcast_to([128, GROUPS, SPG, K])
                )
                out_ap = live.rearrange("p g (s k) -> p g s k", k=K)
                nc.vector.tensor_mul(out_ap, wt_b, sc_b)

            # PE warmup: cold matmul activity releases the HAM throttle
            wtile = psl.tile([MP, GROUPS, CHUNK], f32, tag="logits")
            for _ in range(10):
                nc.tensor.matmul(
                    wtile[:, 0, :], warm[:, 0:128], warm[:, 0:CHUNK],
                    start=True, stop=True,
                )

            totals_sb = totsb.tile([M2, n_st * CHUNK], f32)

            SEL_LAG = 3
            pending_sels = []

            def pop_sel():
                a, k, pack = pending_sels.pop(0)
                nc.tensor.matmul(*a, **k)
                if pack is not None:
                    src_ps, pst_idx, rows, q = pack
                    sl = slice(pst_idx * CHUNK, (pst_idx + 1) * CHUNK)
                    nc.vector.tensor_copy(
                        totals_sb[0:rows, sl], src_ps[0:rows, :]
                    )
                    q.dma_start(out_d[0:rows, sl], totals_sb[0:rows, sl])

            for st, (off, widths) in enumerate(sts):
                w_st = sum(widths)
                if st > 0:
                    f0 = fpool.tile(
                        [128, w_st], f16, tag="f0",
                        padded_shape=[128, SUPER * CHUNK],
                    )
                    f1 = fpool.tile(
                        [128, w_st], f16, tag="f1",
                        padded_shape=[128, SUPER * CHUNK],
                    )
                    if st <= 2:
                        nc.scalar.dma_start(f0[:], fh_d[0:128, off : off + w_st])
                        nc.sync.dma_start(f1[:], fh_d[128:256, off : off + w_st])
                    else:
                        nc.sync.dma_start(f0[:], fh_d[0:128, off : off + w_st])
                        nc.gpsimd.dma_start(f1[:], fh_d[128:256, off : off + w_st])
                else:
                    f0, f1 = f0_first, f1_first

                tot_ps = pst.tile([MP, CHUNK], f32, tag="totals")
                # mid-stream totals go out on sync (a scalar-queue dma_start
                # would stall the ACT instruction stream); the second-to-last
                # supertile uses scalar, which is idle by flush time
                outq = nc.scalar if st == n_st - 2 else nc.sync

                c0 = 0
                for i, cw in enumerate(widths):
                    logits = psl.tile([MP, GROUPS, CHUNK], f32, tag="logits")
                    for g in range(GROUPS):
                        for wo, ww in _bank_windows(g, cw):
                            nc.tensor.matmul(
                                logits[:, g, wo : wo + ww],
                                wall0[:, g, :],
                                f0[:, c0 + wo : c0 + wo + ww],
                                start=True, stop=False,
                            )
                            nc.tensor.matmul(
                                logits[:, g, wo : wo + ww],
                                wall1[:, g, :],
                                f1[:, c0 + wo : c0 + wo + ww],
                                start=False, stop=True,
                            )
                    while len(pending_sels) >= SEL_LAG:
                        pop_sel()

                    sig = sigp.tile([MP, GROUPS, CHUNK], f16, tag="sig")
                    nc.scalar.activation(
                        sig[:, :, 0:cw], logits[:, :, 0:cw], Sig, bias=bias_s[:]
                    )

                    s2 = treep.tile([MP, 2, CHUNK], f16, tag="s2")
                    s4 = treep.tile([MP, CHUNK], f16, tag="s4")
                    s5 = treep.tile([MP, CHUNK], f16, tag="s5")
                    nc.vector.tensor_add(
                        s2[:, :, 0:cw], sig[:, 0:2, 0:cw], sig[:, 2:4, 0:cw]
                    )
                    nc.vector.tensor_add(s4[:, 0:cw], s2[:, 0, 0:cw], s2[:, 1, 0:cw])
                    nc.vector.tensor_add(s5[:, 0:cw], s4[:, 0:cw], sig[:, 4, 0:cw])

                    last = i == len(widths) - 1
                    so = M - 14 * i  # sel_i = sel_s[:, 112-14i : 240-14i]
                    pending_sels.append(
                        (
                            (tot_ps[:, 0:cw], sel_s[:, so : so + MP], s5[:, 0:cw]),
                            dict(start=(i == 0), stop=last),
                            (tot_ps, st, len(widths) * K, outq) if last else None,
                        )
                    )
                    c0 += cw

            while pending_sels:
                pop_sel()
    nc.compile()
    return nc


def _host_inputs(features, weight, bias, masks, hw_total=HW):
    fflat = np.asarray(features, np.float32).reshape(B, C, H * W)[:, :, :hw_total]
    w = np.asarray(weight, np.float32)
    bv = np.asarray(bias, np.float32)
    m = np.asarray(masks)

    # sliding-window selector: sel_i[j, p] = base[j, 112-14i+p]
    # want sel_i[j, 14i + j%14] = 1 (j < 112)  =>  base[j, 112 + j%14] = 1
    j = np.arange(M)
    sel = np.zeros((MP, SELW), np.float16)
    sel[j, M + (j % K)] = 1.0

    biasv = np.zeros((MP, 1), np.float32)
    biasv[:M, 0] = bv[(j % K)]

    wt = w.T.astype(np.float16)  # [C, K]
    in_maps = []
    for b in range(B):
        fh = fflat[b].astype(np.float16)
        scale = (m[:, b, :].astype(np.float32) * 2.0).T.astype(np.float16)  # [C, S]
        wsc = np.concatenate([wt, scale], axis=1)  # [C, K+S]
        in_maps.append({"fh": fh, "wsc": wsc, "sel": sel, "biasv": biasv})
    return in_maps


def _unpack_out(raw):
    """raw [126, n_st*CHUNK] fp32 totals t; host computes the logit of the
    mean: log(t) - log(S - t)."""
    sts = _chunk_layout(HW)
    raw = np.asarray(raw, np.float32).reshape(M2, len(sts) * CHUNK)
    t = np.empty((K, HW), np.float32)
    for st, (off, widths) in enumerate(sts):
        cc = 0
        for i, cw in enumerate(widths):
            t[:, off + cc : off + cc + cw] = raw[
                i * K : (i + 1) * K, st * CHUNK : st * CHUNK + cw
            ]
            cc += cw
    return np.log(t) - np.log(S - t)


def kernel(features, weight, bias, masks):
    from concourse.bass_utils import run_bass_kernel_spmd

    if "nc" not in _CACHE:
        _CACHE["nc"] = _build_program(HW)
    nc = _CACHE["nc"]

    in_maps = _host_inputs(features, weight, bias, masks)
    res = run_bass_kernel_spmd(nc, in_maps, core_ids=list(range(NCORES)))
    out = np.stack(
        [_unpack_out(r["out"]).reshape(K, H, W) for r in res.results], axis=0
    )
    return out.astype(np.float32)
